# revision 15
# baseline (speedup 1.0000x reference)
"""CrossTuckerLayer kernel for 8x Trainium2 NeuronCores (Bass/Tile).

Computes y = einsum('bnvade,ABCDEF,oA,pB,qC,aD,dE,eF->bnvopq', ...)
reshaped to [b, n, v, o*p, q], data-parallel over the 2048 (b,n,v) samples
(256 per core). All HBM I/O is bf16 (harness gate is rel_err < 2e-2; this
path lands ~3.4e-3), halving DMA traffic vs fp32.

Host folds the tiny Tucker factors (all <10K params) into two matrices:
  M    [16384, 8] = einsum('ABCDEF,aD,dE,eF->adeABC', core, a0, a1, a2)
  Wout [8, 32768] = einsum('oA,pB,qC->ABCopq', u0, u1, u2)

Per core the 256 samples split into two 128-sample windows:
  stage A (PE): s2_w[8, 128] = sum over 128 fin-chunks of
      M_ck[128f, 8]^T @ x_ck[128f, 128s].
  s2 is then replicated to partition blocks 0/32/64/96 (one DVE copy +
      three SBUF->SBUF DMAs) so stage C can row-tile the PE.
  stage C (PE): y[128s, 512] tiles = s2_w[8, 128]^T @ W[8, 512] with
      K=8 only — four matmuls run CONCURRENTLY in distinct 32-row PE
      groups via tile_position=(32i, 0). Row-group pairs write the two
      banks of a [128, 1024] PSUM tile, so each PSUM->SBUF copy moves
      1024 cols (1.2ns/col vs 1.6 for 512-col copies; only vector and
      scalar can read PSUM on TRN2). Wout ships as a 0.5MB
      group-permuted [8, 4*8192] tensor, DMA'd into the four 32-row
      SBUF slots via gpsimd SWDGE.

Schedule. The problem is HBM-bound (~8MB x read + 16MB y write + 0.75MB
weights per core). Measured DMA behavior: each descriptor only sustains
~70-80GB/s (engine assignment), so full rate (~400GB/s) needs 5+
concurrent descriptors per direction; service is round-robin, so
same-size descriptors complete together. Hence:
  - x(w0): one tiny 8-chunk tile (stage A starts ~12us in) plus six
    20-chunk (0.625MB) tiles, three per queue, all issued upfront.
  - x(w1): six 20-chunk tiles ring-reusing the w0 buffers (bufs=3per
    queue), so each issue releases mid-stream as stage A consumes the
    matching w0 tile — w0 keeps completion priority, w1 lands during
    the C(w0) copy phase, well before C(w1) needs it.
  - y: per 8192-col stage, two 4096-col half-DMAs (8KB contiguous per
    row), h0 on sync, h1 on scalar; yp bufs=5 so y-DMA completion
    latency never throttles the copies.
  - A(w1) slices are emitted between C(w0) stages 1..3; the PE weaves
    them into the psC-ring idle gaps (psA bufs=2).
"""

import numpy as np
import ml_dtypes

import concourse.bass as bass
import concourse.bacc as bacc
import concourse.mybir as mybir
from concourse.tile import TileContext
from concourse.bass_utils import run_bass_kernel_spmd

F32 = mybir.dt.float32
BF16 = mybir.dt.bfloat16
BF = ml_dtypes.bfloat16

NCORES = 8
S_TOT = 2048          # 4*64*8 samples
S = S_TOT // NCORES   # 256 per core
FIN = 16 * 16 * 64    # 16384
FOUT = 256 * 128      # 32768
NCK = FIN // 128      # 128 contraction chunks of 128
WIN = 128             # samples per window
N_WIN = S // WIN      # 2
YCHUNK = 512          # one matmul's psum cols (fits a 2KB fp32 bank)
YSTAGE = 8192         # cols per y staging tile (two 4096-col DMA halves)
N_YSTAGE = FOUT // YSTAGE  # 4 per window
NTILE = 4             # concurrent row-group matmuls in stage C
NSLOT = FOUT // YCHUNK // NTILE  # 16 column slots per row-group

# x tiles: (engine, window, ck0, nck, tag). Tags ra/rb are 3-deep rings
# (sync / scalar); w1 tiles reuse w0 buffers so their issue releases as
# stage A consumes the matching w0 tile.
X_TILES = [
    ("sync", 0, 0, 8, "x0"),        # starter
    ("sync", 0, 8, 24, "pa"),
    ("scalar", 0, 32, 24, "pb"),
    ("sync", 0, 56, 24, "pa"),
    ("scalar", 0, 80, 24, "pb"),
    ("scalar", 0, 104, 24, "pa"),
    # upfront w1 tiles hold the LAST w1 chunks: they keep queue depth
    # through the w0 tail, but their consumers are late A(w1) matmuls,
    # so the scheduler cannot hoist A(w1) ahead of C(w0) st0.
    ("scalar", 1, 72, 24, "pc"),
    ("sync", 1, 96, 24, "pc"),
    ("sync", 1, 0, 24, "pa"),       # waits x(0,8) consumed
    ("sync", 1, 24, 24, "pb"),      # waits x(0,32) consumed
    ("scalar", 1, 48, 24, "pa"),    # waits x(0,56) consumed
    ("scalar", 1, 120, 8, "x0"),    # waits x(0,0) consumed
]
MM_SPLITS = [(0, 8), (8, 120)]


def _host_weights(core, u0, u1, u2, a0, a1, a2):
    """Fold the Tucker factors into M [128f, 128ck*8] and the
    group-permuted Wout wl_g [8, 4*NSLOT*512]."""
    M = np.einsum(
        "ABCDEF,aD,dE,eF->adeABC",
        core.astype(np.float64), a0.astype(np.float64),
        a1.astype(np.float64), a2.astype(np.float64),
    ).reshape(FIN, 8)
    # SBUF layout [f, ck*8 + r] where fin = ck*128 + f
    Mdev = np.ascontiguousarray(
        M.reshape(NCK, 128, 8).transpose(1, 0, 2).reshape(128, NCK * 8)
    ).astype(BF)

    Wout = np.einsum(
        "oA,pB,qC->ABCopq",
        u0.astype(np.float64), u1.astype(np.float64), u2.astype(np.float64),
    ).reshape(8, FOUT)
    # chunk c of 512 cols -> row-group i = c % 4, col slot j = c // 4;
    # wl_g packs each group's 16 slots contiguously so the device can DMA
    # group i straight into SBUF partitions 32i..32i+8.
    wl_g = np.zeros((8, NTILE * NSLOT * YCHUNK), dtype=np.float64)
    for c in range(FOUT // YCHUNK):
        i, j = c % NTILE, c // NTILE
        wl_g[:, (i * NSLOT + j) * YCHUNK:(i * NSLOT + j + 1) * YCHUNK] = \
            Wout[:, c * YCHUNK:(c + 1) * YCHUNK]
    return Mdev, np.ascontiguousarray(wl_g.astype(BF))


def _host_x(x):
    """x [2048, FIN] f32 -> per-core dev layout [128f, w*16K + ck*128 + s]."""
    xb = x.reshape(S_TOT, FIN).astype(BF)
    xd = np.ascontiguousarray(
        xb.reshape(NCORES, N_WIN, WIN, NCK, 128).transpose(0, 4, 1, 3, 2)
    ).reshape(NCORES, 128, N_WIN * FIN)
    return xd


def _build():
    nc = bacc.Bacc("TRN2", target_bir_lowering=False, debug=False)
    x_d = nc.dram_tensor("x", [128, N_WIN * FIN], BF16, kind="ExternalInput")
    m_d = nc.dram_tensor("m", [128, NCK * 8], BF16, kind="ExternalInput")
    wl_d = nc.dram_tensor("wl", [8, NTILE * NSLOT * YCHUNK], BF16,
                          kind="ExternalInput")
    y_d = nc.dram_tensor("y", [S, FOUT], BF16, kind="ExternalOutput")

    with TileContext(nc) as tc:
        with (
            tc.tile_pool(name="consts", bufs=1) as cpool,
            tc.tile_pool(name="xs", bufs=1) as xs,    # tiny starter ring
            tc.tile_pool(name="xa", bufs=3) as xa,    # 24-chunk ring A
            tc.tile_pool(name="xb", bufs=2) as xb,    # 24-chunk ring B
            tc.tile_pool(name="xc", bufs=2) as xc,    # upfront w1 tiles
            tc.tile_pool(name="s2p", bufs=2) as s2p,
            tc.tile_pool(name="yp", bufs=5) as yp,
            tc.tile_pool(name="psA", bufs=2, space=bass.MemorySpace.PSUM) as psA,
            tc.tile_pool(name="psC", bufs=3, space=bass.MemorySpace.PSUM) as psC,
        ):
            # M slivers so A's first chunks aren't gated on the whole M.
            mm_tiles = []
            for (ck0, n) in MM_SPLITS:
                mmt = cpool.tile([128, n * 8], BF16, name=f"mm_{ck0}")
                nc.scalar.dma_start(mmt[:], m_d[:, ck0 * 8:(ck0 + n) * 8])
                mm_tiles.append((ck0, n, mmt))

            def mm_for(ck):
                for (ck0, n, t) in mm_tiles:
                    if ck0 <= ck < ck0 + n:
                        return t[:, (ck - ck0) * 8:(ck - ck0 + 1) * 8]
                raise AssertionError(ck)

            pools = {"x0": xs, "pa": xa, "pb": xb, "pc": xc}
            x_tiles = {}
            for (eng, w, ck0, n, tag) in X_TILES:
                xg = pools[tag].tile([128, n * WIN], BF16, tag=tag,
                                     name=f"x_{w}_{ck0}")
                getattr(nc, eng).dma_start(
                    xg[:],
                    x_d[:, (w * NCK + ck0) * WIN:(w * NCK + ck0 + n) * WIN],
                )
                x_tiles[(w, ck0)] = xg

            def x_for(w, ck):
                for (eng, ww, ck0, n, tag) in X_TILES:
                    if ww == w and ck0 <= ck < ck0 + n:
                        xg = x_tiles[(w, ck0)]
                        return xg[:, (ck - ck0) * WIN:(ck - ck0 + 1) * WIN]
                raise AssertionError((w, ck))

            # Wout straight into the four 32-row SBUF slots (HWDGE;
            # software DGE stalls the early hardware-queue service).
            wl = cpool.tile([128, NSLOT * YCHUNK], BF16)
            for i in range(NTILE):
                eng = nc.sync if i % 2 == 0 else nc.scalar
                eng.dma_start(
                    wl[32 * i:32 * i + 8, :],
                    wl_d[:, i * NSLOT * YCHUNK:(i + 1) * NSLOT * YCHUNK],
                )

            sA = [psA.tile([8, WIN], F32, tag="sA", name=f"sA_{w}")
                  for w in range(N_WIN)]
            s2r = [s2p.tile([128, WIN], BF16, tag="s2", name=f"s2_{w}")
                   for w in range(N_WIN)]

            def emit_a_slice(w, ck0, n):
                for ck in range(ck0, ck0 + n):
                    nc.tensor.matmul(
                        sA[w][:],
                        mm_for(ck),
                        x_for(w, ck),
                        start=(ck == 0), stop=(ck == NCK - 1),
                        skip_group_check=True,
                    )

            def emit_s2_replicate(w):
                # bf16 downcast into row-group 0, then fan out to 32/64/96
                nc.vector.tensor_copy(s2r[w][0:8, :], sA[w][:])
                nc.sync.dma_start(s2r[w][32:40, :], s2r[w][0:8, :])
                nc.scalar.dma_start(s2r[w][64:72, :], s2r[w][0:8, :])
                nc.sync.dma_start(s2r[w][96:104, :], s2r[w][0:8, :])

            def emit_c_stage(w, st):
                y_sb = yp.tile([128, YSTAGE], BF16, tag="ysb", name="y_sb")
                for jl in range(4):
                    j = st * 4 + jl
                    for ii in range(2):
                        # row-group pair (2ii, 2ii+1) -> one 2-bank tile
                        y_ps = psC.tile([128, 2 * YCHUNK], F32, tag="yps",
                                        name="y_ps")
                        for g in range(2):
                            i = 2 * ii + g
                            nc.tensor.matmul(
                                y_ps[:, g * YCHUNK:(g + 1) * YCHUNK],
                                s2r[w][32 * i:32 * i + 8, :],
                                wl[32 * i:32 * i + 8,
                                   j * YCHUNK:(j + 1) * YCHUNK],
                                start=True, stop=True,
                                tile_position=(32 * i, 0),
                            )
                        dst = y_sb[:, (jl * NTILE + 2 * ii) * YCHUNK:
                                   (jl * NTILE + 2 * ii + 2) * YCHUNK]
                        # alternate engines per pair-tile so the psC ring
                        # turns over at the two engines' combined pace
                        if (2 * jl + ii) % 2 == 0:
                            nc.vector.tensor_copy(dst, y_ps[:])
                        else:
                            nc.scalar.copy(dst, y_ps[:])
                half = 4096
                nc.sync.dma_start(
                    y_d[w * WIN:(w + 1) * WIN,
                        st * YSTAGE:st * YSTAGE + half],
                    y_sb[:, 0:half],
                )
                nc.scalar.dma_start(
                    y_d[w * WIN:(w + 1) * WIN,
                        st * YSTAGE + half:(st + 1) * YSTAGE],
                    y_sb[:, half:YSTAGE],
                )

            # stage A w0 chases the x stream; A(w1) slices interleave
            # with C(w0) stages 1..3.
            for (eng, w, ck0, n, tag) in X_TILES[:6]:
                emit_a_slice(0, ck0, n)
            emit_s2_replicate(0)
            emit_c_stage(0, 0)
            emit_c_stage(0, 1)
            emit_a_slice(1, 0, 24)
            emit_a_slice(1, 24, 24)
            emit_c_stage(0, 2)
            emit_a_slice(1, 48, 24)
            emit_a_slice(1, 72, 24)
            emit_c_stage(0, 3)
            emit_a_slice(1, 96, 24)
            emit_a_slice(1, 120, 8)
            emit_s2_replicate(1)
            for st in range(N_YSTAGE):
                emit_c_stage(1, st)
    nc.compile()
    return nc


_NC_CACHE = []


def _get_nc():
    if not _NC_CACHE:
        _NC_CACHE.append(_build())
    return _NC_CACHE[0]


def run(inputs, trace=False):
    x = np.asarray(inputs["x"], dtype=np.float32)
    Mdev, wl_g = _host_weights(
        np.asarray(inputs["core"]),
        np.asarray(inputs["u0"]), np.asarray(inputs["u1"]),
        np.asarray(inputs["u2"]),
        np.asarray(inputs["a0"]), np.asarray(inputs["a1"]),
        np.asarray(inputs["a2"]),
    )
    xd = _host_x(x)
    nc = _get_nc()
    in_maps = []
    for i in range(NCORES):
        in_maps.append({
            "x": xd[i],
            "m": Mdev,
            "wl": wl_g,
        })
    res = run_bass_kernel_spmd(
        nc, in_maps, core_ids=list(range(NCORES)), trace=trace,
    )
    y = np.concatenate([np.asarray(r["y"]) for r in res.results], axis=0)
    y = y.astype(np.float32).reshape(4, 64, 8, 256, 128)
    return y, res


def kernel(**inputs) -> np.ndarray:
    y, _ = run(inputs, trace=False)
    return y
